# revision 17
# baseline (speedup 1.0000x reference)
"""Trainium2 Bass kernel for nn_GAT_39427799777563 (GAT message passing).

Math (per item row n, K=32 neighbors, D=100 dims):
    We   = entity_embs * w_r                  # [K, D] elementwise
    e_k  = sum_d We[k, d]                     # neighbor logits
    p_k  = softmax_k(leaky_relu(e_k) masked by adj)
    h'   = sum_k p_k * We[k, :]               # weighted neighbor sum
    x    = h' @ W_out.T + b_out + item_embs

v2 design (fp16 streaming):
  - ent/wr cast to fp16 on host: halves HBM traffic (64 MB/core) and puts
    the big DVE tensor_tensor ops in 2x_1p perf mode.
  - J=4 tiles (512 rows) per step amortize per-op fixed costs; inputs are
    pre-swizzled on host so each load is one [128, 12800] per-partition-
    contiguous DMA (25.6 KB runs).
  - We computed in place into the ent tile (SBUF budget).
  - e-sums: fp16 d-folds 100->50->25 (DVE 2x adds) + one strided fp32
    tensor_reduce over 25 for k < M_DVE; the remaining k's accumulate on
    ACT (activation Copy accum_out, ~268 ns each) to balance engines.
  - mask folded as e += adjm (adjm in {0, -60000}) BEFORE leaky_relu: exp
    then yields exactly 0 for masked neighbors, and the softmax denominator
    comes from one small reduce.
  - softmax weights NORMALIZED on chip (pn = exp/sum, fp32) so the weighted
    neighbor values fit fp16.
  - p-apply: DVE broadcast mul (1x) for k < K-Y_ACT, ACT scale-copies
    (scale=pn[:,jk], per-partition fp32 scalar) for the rest - the balance
    knob between the two engines.
  - k-reduction as an in-place fp16 binary tree (5 levels of 2x DVE adds).
  - epilogue per tile: PE transpose -> ACT psum copy (fp16) -> PE matmul
    with W_out.T fp16 -> DVE add of (item_embs + b_out) fp32 residual.

Sharding: pure data parallel over N across 8 cores; rows padded
40000 -> 40960 so every core runs 10 groups of 4 128-row tiles.
"""

from contextlib import ExitStack
import os as _os

import numpy as np

import concourse.bass as bass
import concourse.bacc as bacc
import concourse.mybir as mybir
import concourse.tile as tile

F32 = mybir.dt.float32
F16 = mybir.dt.float16
ALPHA = 0.2
MASK_NEG = -60000.0

N, K, D = 40000, 32, 100
N_CORES = 8
P = 128
J = int(_os.environ.get("GAT_J", "4"))        # tiles per group
M_DVE = int(_os.environ.get("GAT_M_DVE", "32"))   # k's e-summed on DVE
Y_ACT = int(_os.environ.get("GAT_Y_ACT", "20"))   # k's p-scaled on ACT
ENT_BUFS = int(_os.environ.get("GAT_ENT_BUFS", "4"))
TREE_DMA = int(_os.environ.get("GAT_TREE_DMA", "2"))  # tree levels on SWDGE
STORE_CHUNK = 8    # tiles per output store


def build(n_tiles: int, repeats: int = 1, mode: str = "full"):
    """Build the per-core Bass program for n_tiles 128-row tiles.

    repeats > 1 wraps the whole loop in a hardware For_i loop that
    re-executes it, for dispatch-overhead-free benchmarking.
    mode: "full" | "dma" (DMA-only ablation)."""
    assert n_tiles % J == 0
    G = n_tiles // J
    F = J * K * D
    nc = bacc.Bacc("TRN2", target_bir_lowering=False, debug=False,
                   num_devices=N_CORES)

    ent_d = nc.dram_tensor("ent", [P, G * F], F16, kind="ExternalInput")
    wr_d = nc.dram_tensor("wr", [P, G * F], F16, kind="ExternalInput")
    adjm_d = nc.dram_tensor("adjm", [P, G * J * K], F16, kind="ExternalInput")
    itemb_d = nc.dram_tensor("itemb", [P, G * J * D], F32, kind="ExternalInput")
    wt_d = nc.dram_tensor("wt", [D, D], F16, kind="ExternalInput")   # W_out.T
    ident_d = nc.dram_tensor("ident", [P, P], F16, kind="ExternalInput")
    out_d = nc.dram_tensor("out", [P, G * J * D], F32, kind="ExternalOutput")

    with tile.TileContext(nc) as tc, ExitStack() as ctx:
        const = ctx.enter_context(tc.tile_pool(name="const", bufs=1))
        big = ctx.enter_context(tc.tile_pool(name="big", bufs=ENT_BUFS))
        wrp = ctx.enter_context(tc.tile_pool(name="wrp", bufs=2))
        small = ctx.enter_context(tc.tile_pool(name="small", bufs=2))
        psum = ctx.enter_context(tc.tile_pool(name="psum", bufs=4, space="PSUM"))

        adjm = const.tile([P, G * J * K], F16)
        itemb = const.tile([P, G * J * D], F32)
        wt = const.tile([D, D], F16)
        ident = const.tile([P, P], F16)
        out_all = const.tile([P, G * J * D], F32)
        def emit_consts():
            nc.sync.dma_start(adjm[:], adjm_d[:])
            nc.sync.dma_start(itemb[:], itemb_d[:])
            nc.sync.dma_start(wt[:], wt_d[:])
            nc.sync.dma_start(ident[:], ident_d[:])

        def loop():
            body(nc, tc, G, ent_d, wr_d, out_d, adjm, itemb, wt, ident,
                 out_all, big, wrp, small, psum, mode=mode,
                 emit_consts=emit_consts)

        if repeats > 1:
            with tc.For_i(0, repeats, 1):
                loop()
        else:
            loop()

    nc.compile()
    return nc


def body(nc, tc, G, ent_d, wr_d, out_d, adjm, itemb, wt, ident,
         out_all, big, wrp, small, psum, mode="full", emit_consts=None):
    AF = mybir.ActivationFunctionType
    AL = mybir.AluOpType
    AX = mybir.AxisListType
    F = J * K * D
    SC = STORE_CHUNK // J   # groups per output store

    if mode == "dma":
        for g in range(G):
            ent_t = big.tile([P, F], F16, tag="ent")
            nc.sync.dma_start(ent_t[:], ent_d[:, g * F:(g + 1) * F])
            wr_t = wrp.tile([P, F], F16, tag="wr")
            nc.sync.dma_start(wr_t[:], wr_d[:, g * F:(g + 1) * F])
            for j in range(J):
                t = g * J + j
                nc.vector.tensor_copy(out_all[:, t * D:(t + 1) * D],
                                      ent_t[:, j * K * D:j * K * D + D])
            if (g + 1) % SC == 0:
                csl = slice((g + 1 - SC) * J * D, (g + 1) * J * D)
                nc.sync.dma_start(out_d[:, csl], out_all[:, csl])
        rem = G % SC
        if rem:
            csl = slice((G - rem) * J * D, G * J * D)
            nc.sync.dma_start(out_d[:, csl], out_all[:, csl])
        return

    def front(g):
        """Load + We + e-sums + softmax smalls + p-apply for group g."""
        ent_t = big.tile([P, F], F16, tag="ent")
        nc.sync.dma_start(ent_t[:], ent_d[:, g * F:(g + 1) * F])
        wr_t = wrp.tile([P, F], F16, tag="wr")
        if g == 0:
            # group 0 only: wr rides the (idle) ACT HWDGE ring so both
            # input streams land in parallel and the first We starts ~10us
            # earlier; steady-state groups keep SP to avoid ACT queue stalls
            nc.scalar.dma_start(wr_t[:], wr_d[:, g * F:(g + 1) * F])
        else:
            nc.sync.dma_start(wr_t[:], wr_d[:, g * F:(g + 1) * F])
        if g == 0 and emit_consts is not None:
            emit_consts()  # const loads queue on SP behind group 0's loads

        # We = ent * wr, in place into the ent tile (DVE, fp16 2x)
        we = ent_t
        nc.vector.tensor_mul(we[:], ent_t[:], wr_t[:])
        we4 = we[:].rearrange("p (j k d) -> p j k d", j=J, k=K)

        # e_k = sum_d We[k, :]
        e = small.tile([P, J * K], F32, tag="e")
        m = M_DVE
        if m > 0:
            # wr_t is dead after the We product - reuse it as fold scratch
            t13 = wr_t[:, :J * m * 50].rearrange("p (j k d) -> p j k d",
                                                 j=J, k=m)
            nc.vector.tensor_add(t13, we4[:, :, :m, :50], we4[:, :, :m, 50:])
            t23 = wr_t[:, J * m * 50:J * m * 75].rearrange(
                "p (j k d) -> p j k d", j=J, k=m)
            nc.vector.tensor_add(t23, t13[:, :, :, :25], t13[:, :, :, 25:])
            nc.vector.tensor_reduce(
                e[:].rearrange("p (j k) -> p j k", j=J)[:, :, :m], t23,
                axis=AX.X, op=AL.add)
        for j in range(J):
            for k in range(m, K):
                sl = slice(j * K * D + k * D, j * K * D + (k + 1) * D)
                nc.scalar.activation(we[:, sl], we[:, sl], AF.Copy,
                                     accum_out=e[:, j * K + k:j * K + k + 1])

        # mask (+-60000 bias) then leaky relu, in place (DVE)
        nc.vector.tensor_add(e[:], e[:], adjm[:, g * J * K:(g + 1) * J * K])
        nc.vector.scalar_tensor_tensor(e[:], e[:], ALPHA, e[:],
                                       op0=AL.mult, op1=AL.max)

        # exp (ACT), denominator + normalized weights (DVE, fp32)
        ex = small.tile([P, J * K], F32, tag="ex")
        nc.scalar.activation(ex[:], e[:], AF.Exp)
        sumexp = small.tile([P, J], F32, tag="sumexp")
        nc.vector.tensor_reduce(sumexp[:],
                                ex[:].rearrange("p (j k) -> p j k", j=J),
                                axis=AX.X, op=AL.add)
        rs = small.tile([P, J], F32, tag="rs")
        nc.vector.reciprocal(rs[:], sumexp[:])
        pn = small.tile([P, J * K], F32, tag="pn")
        nc.vector.tensor_mul(
            pn[:].rearrange("p (j k) -> p j k", j=J),
            ex[:].rearrange("p (j k) -> p j k", j=J),
            rs[:].unsqueeze(-1).broadcast_to([P, J, K]))

        return we4, pn

    def tail(g, we4, pn):
        """Per-j: p-apply + k-tree + linear epilogue for group g."""
        we = we4.rearrange("p j k d -> p (j k d)")
        kd = K - Y_ACT
        for j in range(J):
            t = g * J + j
            # p-apply for this j: DVE broadcast mul + ACT scale-copies
            if kd > 0:
                pn3 = pn[:].rearrange("p (j k) -> p j k", j=J)[:, j:j+1, :kd]
                nc.vector.tensor_mul(
                    we4[:, j:j+1, :kd, :], we4[:, j:j+1, :kd, :],
                    pn3.unsqueeze(-1).broadcast_to([P, 1, kd, D]))
            for k in range(kd, K):
                sl = slice(j * K * D + k * D, j * K * D + (k + 1) * D)
                nc.scalar.activation(we[:, sl], we[:, sl], AF.Copy,
                                     scale=pn[:, j * K + k:j * K + k + 1])
            # k-tree for this j
            base = j * K * D
            kk = K
            lvl = 0
            while kk > 4:
                h = kk // 2
                lo = we[:, base:base + h * D]
                hi = we[:, base + h * D:base + kk * D]
                if lvl < TREE_DMA:
                    with nc.allow_low_precision(reason="fp16 tree verified"):
                        nc.gpsimd.dma_start(lo, hi, accum_op=AL.add)
                else:
                    nc.vector.tensor_add(lo, lo, hi)
                kk = h
                lvl += 1
            hu_t = small.tile([P, D], F16, tag="hu")
            with nc.allow_low_precision(reason="fp16 k-sum verified vs gate"):
                nc.vector.tensor_reduce(
                    hu_t[:].unsqueeze(1),
                    we4[:, j:j+1, :kk, :].rearrange("p j k d -> p j d k"),
                    axis=AX.X, op=AL.add)
            ht_ps = psum.tile([D, P], F16, tag="htp")
            nc.tensor.transpose(ht_ps[:], hu_t[:], ident[:])
            ht = small.tile([D, P], F16, tag="ht")
            nc.scalar.copy(ht[:], ht_ps[:])
            x_ps = psum.tile([P, D], F32, tag="xps")
            nc.tensor.matmul(x_ps[:], ht[:], wt[:], start=True, stop=True)
            nc.vector.tensor_add(out_all[:, t * D:(t + 1) * D], x_ps[:],
                                 itemb[:, t * D:(t + 1) * D])

    # software pipeline: emit front(g+1) before tail(g) so the scheduler
    # keeps ACT fed with the next group's scale-copies while DVE drains
    # the previous group's tree.
    half = (G // 2) * J * D
    pend = None
    for g in range(G):
        we4pn = front(g)
        if pend is not None:
            tail(pend[0], *pend[1])
            if pend[0] == G // 2:
                nc.sync.dma_start(out_d[:, :half], out_all[:, :half])
        pend = (g, we4pn)
    tail(pend[0], *pend[1])
    # out_all is resident; the tail store covers the back half (the front
    # half was stored mid-loop above).
    nc.sync.dma_start(out_d[:, half:], out_all[:, half:])


def _shard_host(item_embs, entity_embs, w_r, adj, W_out, b_out, n_tiles):
    """Pad + shard + swizzle the full inputs into 8 per-core input maps."""
    G = n_tiles // J
    rows = n_tiles * P
    n_pad = N_CORES * rows

    ent = np.asarray(entity_embs, np.float16).reshape(N, K * D)
    wr = np.asarray(w_r, np.float16).reshape(N, K * D)
    adjm = np.where(np.asarray(adj) > 0, 0.0, MASK_NEG).astype(np.float16)
    itemb = np.asarray(item_embs, np.float32) + np.asarray(b_out, np.float32)

    pad = n_pad - N
    ent = np.pad(ent, ((0, pad), (0, 0)))
    wr = np.pad(wr, ((0, pad), (0, 0)))
    # padding rows keep adjm=0 (mask open) so the denominator stays nonzero
    adjm = np.pad(adjm, ((0, pad), (0, 0)))
    itemb = np.pad(itemb, ((0, pad), (0, 0)))

    wt = np.ascontiguousarray(np.asarray(W_out, np.float16).T)
    ident = np.eye(P, dtype=np.float16)

    def swz(a, width):
        # [rows, width] -> [P, G * J * width], row (g, j, p) contiguous per p
        return np.ascontiguousarray(
            a.reshape(G, J, P, width).transpose(2, 0, 1, 3)
            .reshape(P, G * J * width))

    in_maps = []
    for c in range(N_CORES):
        rsl = slice(c * rows, (c + 1) * rows)
        in_maps.append({
            "ent": swz(ent[rsl], K * D),
            "wr": swz(wr[rsl], K * D),
            "adjm": swz(adjm[rsl], K),
            "itemb": swz(itemb[rsl], D),
            "wt": wt,
            "ident": ident,
        })
    return in_maps


def _unshard_host(results, n_tiles):
    G = n_tiles // J
    rows = n_tiles * P
    outs = []
    for c in range(N_CORES):
        o = results[c]["out"]  # [P, G * J * D] swizzled
        outs.append(o.reshape(P, G, J, D).transpose(1, 2, 0, 3)
                    .reshape(rows, D))
    return np.concatenate(outs)[:N]


_N_TILES_FULL = 40  # 8 cores * 40 tiles * 128 rows = 40960 >= 40000


def kernel(item_embs, entity_embs, w_r, adj, W_out, b_out):
    from concourse.bass_utils import run_bass_kernel_spmd

    nc = build(_N_TILES_FULL)
    in_maps = _shard_host(item_embs, entity_embs, w_r, adj, W_out, b_out,
                          _N_TILES_FULL)
    res = run_bass_kernel_spmd(nc, in_maps, core_ids=list(range(N_CORES)))
    return _unshard_host(res.results, _N_TILES_FULL).astype(np.float32)


# revision 19
# speedup vs baseline: 1.3711x; 1.3711x over previous
"""Trainium2 Bass kernel for nn_GAT_39427799777563 (GAT message passing).

Math (per item row n, K=32 neighbors, D=100 dims):
    We   = entity_embs * w_r                  # [K, D] elementwise
    e_k  = sum_d We[k, d]                     # neighbor logits
    p_k  = softmax_k(leaky_relu(e_k) masked by adj)
    h'   = sum_k p_k * We[k, :]               # weighted neighbor sum
    x    = h' @ W_out.T + b_out + item_embs

v2 design (fp16 streaming):
  - ent/wr cast to fp16 on host: halves HBM traffic (64 MB/core) and puts
    the big DVE tensor_tensor ops in 2x_1p perf mode.
  - J=4 tiles (512 rows) per step amortize per-op fixed costs; inputs are
    pre-swizzled on host so each load is one [128, 12800] per-partition-
    contiguous DMA (25.6 KB runs).
  - We computed in place into the ent tile (SBUF budget).
  - e-sums: fp16 d-folds 100->50->25 (DVE 2x adds) + one strided fp32
    tensor_reduce over 25 for k < M_DVE; the remaining k's accumulate on
    ACT (activation Copy accum_out, ~268 ns each) to balance engines.
  - mask folded as e += adjm (adjm in {0, -60000}) BEFORE leaky_relu: exp
    then yields exactly 0 for masked neighbors, and the softmax denominator
    comes from one small reduce.
  - softmax weights NORMALIZED on chip (pn = exp/sum, fp32) so the weighted
    neighbor values fit fp16.
  - p-apply: DVE broadcast mul (1x) for k < K-Y_ACT, ACT scale-copies
    (scale=pn[:,jk], per-partition fp32 scalar) for the rest - the balance
    knob between the two engines.
  - k-reduction as an in-place fp16 binary tree (5 levels of 2x DVE adds).
  - epilogue per tile: PE transpose -> ACT psum copy (fp16) -> PE matmul
    with W_out.T fp16 -> DVE add of (item_embs + b_out) fp32 residual.

Sharding: pure data parallel over N across 8 cores; rows padded
40000 -> 40960 so every core runs 10 groups of 4 128-row tiles.
"""

from contextlib import ExitStack
import os as _os

import numpy as np

import concourse.bass as bass
import concourse.bacc as bacc
import concourse.mybir as mybir
import concourse.tile as tile

F32 = mybir.dt.float32
F16 = mybir.dt.float16
ALPHA = 0.2
MASK_NEG = -60000.0

N, K, D = 40000, 32, 100
N_CORES = 8
P = 128
J = int(_os.environ.get("GAT_J", "4"))        # tiles per group
M_DVE = int(_os.environ.get("GAT_M_DVE", "32"))   # k's e-summed on DVE
Y_ACT = int(_os.environ.get("GAT_Y_ACT", "24"))   # k's p-scaled on ACT
ENT_BUFS = int(_os.environ.get("GAT_ENT_BUFS", "4"))
TREE_DMA = int(_os.environ.get("GAT_TREE_DMA", "0"))  # tree levels on SWDGE
STORE_CHUNK = 8    # tiles per output store


def build(n_tiles: int, repeats: int = 1, mode: str = "full"):
    """Build the per-core Bass program for n_tiles 128-row tiles.

    repeats > 1 wraps the whole loop in a hardware For_i loop that
    re-executes it, for dispatch-overhead-free benchmarking.
    mode: "full" | "dma" (DMA-only ablation)."""
    assert n_tiles % J == 0
    G = n_tiles // J
    F = J * K * D
    nc = bacc.Bacc("TRN2", target_bir_lowering=False, debug=False,
                   num_devices=N_CORES)

    ent_d = nc.dram_tensor("ent", [P, G * F], F16, kind="ExternalInput")
    wr_d = nc.dram_tensor("wr", [P, G * F], F16, kind="ExternalInput")
    adjm_d = nc.dram_tensor("adjm", [P, G * J * K], F16, kind="ExternalInput")
    itemb_d = nc.dram_tensor("itemb", [P, G * J * D], F32, kind="ExternalInput")
    wt_d = nc.dram_tensor("wt", [D, D], F16, kind="ExternalInput")   # W_out.T
    ident_d = nc.dram_tensor("ident", [P, P], F16, kind="ExternalInput")
    out_d = nc.dram_tensor("out", [P, G * J * D], F32, kind="ExternalOutput")

    with tile.TileContext(nc) as tc, ExitStack() as ctx:
        const = ctx.enter_context(tc.tile_pool(name="const", bufs=1))
        big = ctx.enter_context(tc.tile_pool(name="big", bufs=ENT_BUFS))
        wrp = ctx.enter_context(tc.tile_pool(name="wrp", bufs=2))
        small = ctx.enter_context(tc.tile_pool(name="small", bufs=2))
        psum = ctx.enter_context(tc.tile_pool(name="psum", bufs=4, space="PSUM"))

        adjm = const.tile([P, G * J * K], F16)
        itemb = const.tile([P, G * J * D], F32)
        wt = const.tile([D, D], F16)
        ident = const.tile([P, P], F16)
        out_all = const.tile([P, G * J * D], F32)
        def emit_consts():
            nc.sync.dma_start(adjm[:], adjm_d[:])
            nc.sync.dma_start(itemb[:], itemb_d[:])
            nc.sync.dma_start(wt[:], wt_d[:])
            nc.sync.dma_start(ident[:], ident_d[:])

        def loop():
            body(nc, tc, G, ent_d, wr_d, out_d, adjm, itemb, wt, ident,
                 out_all, big, wrp, small, psum, mode=mode,
                 emit_consts=emit_consts)

        if repeats > 1:
            with tc.For_i(0, repeats, 1):
                loop()
        else:
            loop()

    nc.compile()
    return nc


def body(nc, tc, G, ent_d, wr_d, out_d, adjm, itemb, wt, ident,
         out_all, big, wrp, small, psum, mode="full", emit_consts=None):
    AF = mybir.ActivationFunctionType
    AL = mybir.AluOpType
    AX = mybir.AxisListType
    F = J * K * D
    SC = STORE_CHUNK // J   # groups per output store

    if mode == "dma":
        for g in range(G):
            ent_t = big.tile([P, F], F16, tag="ent")
            nc.sync.dma_start(ent_t[:], ent_d[:, g * F:(g + 1) * F])
            wr_t = wrp.tile([P, F], F16, tag="wr")
            nc.sync.dma_start(wr_t[:], wr_d[:, g * F:(g + 1) * F])
            for j in range(J):
                t = g * J + j
                nc.vector.tensor_copy(out_all[:, t * D:(t + 1) * D],
                                      ent_t[:, j * K * D:j * K * D + D])
            if (g + 1) % SC == 0:
                csl = slice((g + 1 - SC) * J * D, (g + 1) * J * D)
                nc.sync.dma_start(out_d[:, csl], out_all[:, csl])
        rem = G % SC
        if rem:
            csl = slice((G - rem) * J * D, G * J * D)
            nc.sync.dma_start(out_d[:, csl], out_all[:, csl])
        return

    def front(g):
        """Load + We + e-sums + softmax smalls + p-apply for group g."""
        ent_t = big.tile([P, F], F16, tag="ent")
        nc.sync.dma_start(ent_t[:], ent_d[:, g * F:(g + 1) * F])
        wr_t = wrp.tile([P, F], F16, tag="wr")
        if g == 0:
            # group 0 only: wr rides the (idle) ACT HWDGE ring so both
            # input streams land in parallel and the first We starts ~10us
            # earlier; steady-state groups keep SP to avoid ACT queue stalls
            nc.scalar.dma_start(wr_t[:], wr_d[:, g * F:(g + 1) * F])
        else:
            nc.sync.dma_start(wr_t[:], wr_d[:, g * F:(g + 1) * F])
        if g == 0 and emit_consts is not None:
            emit_consts()  # const loads queue on SP behind group 0's loads

        # We = ent * wr, in place into the ent tile (DVE, fp16 2x)
        we = ent_t
        nc.vector.tensor_mul(we[:], ent_t[:], wr_t[:])
        we4 = we[:].rearrange("p (j k d) -> p j k d", j=J, k=K)

        # e_k = sum_d We[k, :]
        e = small.tile([P, J * K], F32, tag="e")
        m = M_DVE
        if m > 0:
            # wr_t is dead after the We product - reuse it as fold scratch
            t13 = wr_t[:, :J * m * 50].rearrange("p (j k d) -> p j k d",
                                                 j=J, k=m)
            nc.vector.tensor_add(t13, we4[:, :, :m, :50], we4[:, :, :m, 50:])
            t23 = wr_t[:, J * m * 50:J * m * 75].rearrange(
                "p (j k d) -> p j k d", j=J, k=m)
            nc.vector.tensor_add(t23, t13[:, :, :, :25], t13[:, :, :, 25:])
            nc.vector.tensor_reduce(
                e[:].rearrange("p (j k) -> p j k", j=J)[:, :, :m], t23,
                axis=AX.X, op=AL.add)
        for j in range(J):
            for k in range(m, K):
                sl = slice(j * K * D + k * D, j * K * D + (k + 1) * D)
                nc.scalar.activation(we[:, sl], we[:, sl], AF.Copy,
                                     accum_out=e[:, j * K + k:j * K + k + 1])

        # mask (+-60000 bias) then leaky relu, in place (DVE)
        nc.vector.tensor_add(e[:], e[:], adjm[:, g * J * K:(g + 1) * J * K])
        nc.vector.scalar_tensor_tensor(e[:], e[:], ALPHA, e[:],
                                       op0=AL.mult, op1=AL.max)

        # exp (ACT), denominator + normalized weights (DVE, fp32)
        ex = small.tile([P, J * K], F32, tag="ex")
        nc.scalar.activation(ex[:], e[:], AF.Exp)
        sumexp = small.tile([P, J], F32, tag="sumexp")
        nc.vector.tensor_reduce(sumexp[:],
                                ex[:].rearrange("p (j k) -> p j k", j=J),
                                axis=AX.X, op=AL.add)
        rs = small.tile([P, J], F32, tag="rs")
        nc.vector.reciprocal(rs[:], sumexp[:])
        pn = small.tile([P, J * K], F32, tag="pn")
        nc.vector.tensor_mul(
            pn[:].rearrange("p (j k) -> p j k", j=J),
            ex[:].rearrange("p (j k) -> p j k", j=J),
            rs[:].unsqueeze(-1).broadcast_to([P, J, K]))

        return we4, pn

    def tail(g, we4, pn):
        """p-apply (per-j) + group k-tree + per-j linear epilogue."""
        we = we4.rearrange("p j k d -> p (j k d)")
        kd = K - Y_ACT
        for j in range(J):
            if kd > 0:
                pn3 = pn[:].rearrange("p (j k) -> p j k", j=J)[:, j:j+1, :kd]
                nc.vector.tensor_mul(
                    we4[:, j:j+1, :kd, :], we4[:, j:j+1, :kd, :],
                    pn3.unsqueeze(-1).broadcast_to([P, 1, kd, D]))
            for k in range(kd, K):
                sl = slice(j * K * D + k * D, j * K * D + (k + 1) * D)
                nc.scalar.activation(we[:, sl], we[:, sl], AF.Copy,
                                     scale=pn[:, j * K + k:j * K + k + 1])
        # group-wide tree to kk=4 (big 2x DVE adds spanning all j)
        kk = K
        while kk > 4:
            h = kk // 2
            nc.vector.tensor_add(we4[:, :, :h, :], we4[:, :, :h, :],
                                 we4[:, :, h:kk, :])
            kk = h
        # per-j: final strided reduce + transpose + linear + residual
        for j in range(J):
            t = g * J + j
            hu_t = small.tile([P, D], F16, tag="hu")
            with nc.allow_low_precision(reason="fp16 k-sum verified vs gate"):
                nc.vector.tensor_reduce(
                    hu_t[:].unsqueeze(1),
                    we4[:, j:j+1, :kk, :].rearrange("p j k d -> p j d k"),
                    axis=AX.X, op=AL.add)
            ht_ps = psum.tile([D, P], F16, tag="htp")
            nc.tensor.transpose(ht_ps[:], hu_t[:], ident[:])
            ht = small.tile([D, P], F16, tag="ht")
            nc.scalar.copy(ht[:], ht_ps[:])
            x_ps = psum.tile([P, D], F32, tag="xps")
            nc.tensor.matmul(x_ps[:], ht[:], wt[:], start=True, stop=True)
            nc.vector.tensor_add(out_all[:, t * D:(t + 1) * D], x_ps[:],
                                 itemb[:, t * D:(t + 1) * D])

    # software pipeline: emit front(g+1) before tail(g) so the scheduler
    # keeps ACT fed with the next group's scale-copies while DVE drains
    # the previous group's tree.
    half = (G // 2) * J * D
    pend = None
    for g in range(G):
        we4pn = front(g)
        if pend is not None:
            tail(pend[0], *pend[1])
            if pend[0] == G // 2:
                nc.sync.dma_start(out_d[:, :half], out_all[:, :half])
        pend = (g, we4pn)
    tail(pend[0], *pend[1])
    # out_all is resident; the tail store covers the back half (the front
    # half was stored mid-loop above).
    nc.sync.dma_start(out_d[:, half:], out_all[:, half:])


def _shard_host(item_embs, entity_embs, w_r, adj, W_out, b_out, n_tiles):
    """Pad + shard + swizzle the full inputs into 8 per-core input maps."""
    G = n_tiles // J
    rows = n_tiles * P
    n_pad = N_CORES * rows

    ent = np.asarray(entity_embs, np.float16).reshape(N, K * D)
    wr = np.asarray(w_r, np.float16).reshape(N, K * D)
    adjm = np.where(np.asarray(adj) > 0, 0.0, MASK_NEG).astype(np.float16)
    itemb = np.asarray(item_embs, np.float32) + np.asarray(b_out, np.float32)

    pad = n_pad - N
    ent = np.pad(ent, ((0, pad), (0, 0)))
    wr = np.pad(wr, ((0, pad), (0, 0)))
    # padding rows keep adjm=0 (mask open) so the denominator stays nonzero
    adjm = np.pad(adjm, ((0, pad), (0, 0)))
    itemb = np.pad(itemb, ((0, pad), (0, 0)))

    wt = np.ascontiguousarray(np.asarray(W_out, np.float16).T)
    ident = np.eye(P, dtype=np.float16)

    def swz(a, width):
        # [rows, width] -> [P, G * J * width], row (g, j, p) contiguous per p
        return np.ascontiguousarray(
            a.reshape(G, J, P, width).transpose(2, 0, 1, 3)
            .reshape(P, G * J * width))

    in_maps = []
    for c in range(N_CORES):
        rsl = slice(c * rows, (c + 1) * rows)
        in_maps.append({
            "ent": swz(ent[rsl], K * D),
            "wr": swz(wr[rsl], K * D),
            "adjm": swz(adjm[rsl], K),
            "itemb": swz(itemb[rsl], D),
            "wt": wt,
            "ident": ident,
        })
    return in_maps


def _unshard_host(results, n_tiles):
    G = n_tiles // J
    rows = n_tiles * P
    outs = []
    for c in range(N_CORES):
        o = results[c]["out"]  # [P, G * J * D] swizzled
        outs.append(o.reshape(P, G, J, D).transpose(1, 2, 0, 3)
                    .reshape(rows, D))
    return np.concatenate(outs)[:N]


_N_TILES_FULL = 40  # 8 cores * 40 tiles * 128 rows = 40960 >= 40000


def kernel(item_embs, entity_embs, w_r, adj, W_out, b_out):
    from concourse.bass_utils import run_bass_kernel_spmd

    nc = build(_N_TILES_FULL)
    in_maps = _shard_host(item_embs, entity_embs, w_r, adj, W_out, b_out,
                          _N_TILES_FULL)
    res = run_bass_kernel_spmd(nc, in_maps, core_ids=list(range(N_CORES)))
    return _unshard_host(res.results, _N_TILES_FULL).astype(np.float32)
